# revision 10
# baseline (speedup 1.0000x reference)
"""Trainium2 Bass kernel for nn_CustomLinear (learned-twiddle butterfly net).

Math (verified vs reference in fp32, rel err ~4e-7):
  reference pads x [2048,4096] to [2048,8192], half-swaps (XOR N/2), then 13
  radix-2 butterfly stages with learned twiddles tw_s[i] = exp(-2j*pi/N *
  k*w[k]), k = i*N/step.  After the half-swap the first 4096 elements of each
  row are ZERO, so:
    - stages 1..7  == one 128x128 complex matrix M applied to each of the 32
      nonzero 128-blocks (input real -> 2 real matmuls)
    - stages 8..12 == butterflies on the 4096-element nonzero subvector
    - stage 13     == out = [t, -t],  t = tw13 * v   (lo half is zero)

Device layout: transposed blocks [e=partition, (j,r)=free] so twiddles are
per-partition scalars (scalar_tensor_tensor / ACT scale).  PE does the block
transposes in/out and the M matmuls; DVE/ACT/GPSIMD share butterflies+copies.

Sharding: pure data parallel, batch 2048 -> 8 cores x 256 rows.
"""
import numpy as np
from contextlib import ExitStack

import concourse.bacc as bacc
import concourse.mybir as mybir
from concourse.tile import TileContext
from concourse.bass_utils import run_bass_kernel_spmd

N = 8192
B = 2048
IN_F = 4096
NCORES = 8
B_CORE = B // NCORES          # 256 rows per core
NTILES = B_CORE // 128        # 2 row-tiles of 128 rows
NBLK = 32                     # nonzero 128-blocks per row
F32 = mybir.dt.float32

# const tensor column layout
_MRE, _MIM, _ID = 0, 128, 256
_TWR, _TWI = 384, 415         # 31 cols each (stages 8..12)
_T13C, _T13S = 446, 478       # 32 cols each
CW_W = 512

_CACHE = {}


def _stage_tw(s, w):
    step = 1 << s
    half = step >> 1
    k = np.arange(half) * (N // step)
    ang = (-2.0 * np.pi / N) * k.astype(np.float64) * w[k].astype(np.float64)
    return np.exp(1j * ang)


def _host_consts(w):
    M = np.eye(128, dtype=np.complex128)
    for s in range(1, 8):
        step = 1 << s
        half = step >> 1
        tw = _stage_tw(s, w)
        Bm = np.zeros((step, step), np.complex128)
        Bm[:half, :half] = np.eye(half)
        Bm[:half, half:] = np.diag(tw)
        Bm[half:, :half] = np.eye(half)
        Bm[half:, half:] = -np.diag(tw)
        M = np.kron(np.eye(128 // step), Bm) @ M

    cw = np.zeros((128, CW_W), np.float32)
    cw[:, _MRE:_MRE + 128] = M.real.T.astype(np.float32)
    cw[:, _MIM:_MIM + 128] = M.imag.T.astype(np.float32)
    cw[:, _ID:_ID + 128] = np.eye(128, dtype=np.float32)
    off = 0
    for s in range(8, 13):
        tw = _stage_tw(s, w)
        hb = 1 << (s - 8)           # hi width in blocks
        for jr in range(hb):
            cw[:, _TWR + off] = tw.real[jr * 128:(jr + 1) * 128]
            cw[:, _TWI + off] = tw.imag[jr * 128:(jr + 1) * 128]
            off += 1
    tw13 = _stage_tw(13, w)
    for j in range(NBLK):
        cw[:, _T13C + j] = tw13.real[j * 128:(j + 1) * 128]
        cw[:, _T13S + j] = tw13.imag[j * 128:(j + 1) * 128]
    return cw


def _build_program():
    nc = bacc.Bacc("TRN2", target_bir_lowering=False, debug=False)
    x_d = nc.dram_tensor("x", [B_CORE, IN_F], F32, kind="ExternalInput").ap()
    cw_d = nc.dram_tensor("cw", [128, CW_W], F32, kind="ExternalInput").ap()
    y_d = nc.dram_tensor("y", [B_CORE, 2 * N], F32, kind="ExternalOutput").ap()

    AL = mybir.AluOpType

    with TileContext(nc) as tc, ExitStack() as ctx:
        cpool = ctx.enter_context(tc.tile_pool(name="const", bufs=1))
        xpool = ctx.enter_context(tc.tile_pool(name="xin", bufs=1))
        xtpool = ctx.enter_context(tc.tile_pool(name="xt", bufs=2))
        tpool = ctx.enter_context(tc.tile_pool(name="t13", bufs=1))
        zpool = ctx.enter_context(tc.tile_pool(name="z", bufs=1))
        spool = ctx.enter_context(tc.tile_pool(name="scr", bufs=2))
        opool = ctx.enter_context(tc.tile_pool(name="out", bufs=3))
        ps_t = ctx.enter_context(tc.tile_pool(name="ps_t", bufs=2, space="PSUM"))
        ps_m = ctx.enter_context(tc.tile_pool(name="ps_m", bufs=2, space="PSUM"))
        ps_o = ctx.enter_context(tc.tile_pool(name="ps_o", bufs=2, space="PSUM"))

        cw = cpool.tile([128, CW_W], F32)
        nc.sync.dma_start(cw[:], cw_d[:])
        ident = cw[:, _ID:_ID + 128]
        mre_t = cw[:, _MRE:_MRE + 128]
        mim_t = cw[:, _MIM:_MIM + 128]

        for ti in range(NTILES):
            r0 = ti * 128
            # ---- load + transpose 32 blocks:  XT[e, j*128+r] ----
            xin = xpool.tile([128, IN_F], F32, tag="xin")
            nc.sync.dma_start(xin[:], x_d[r0:r0 + 128, :])
            xt = xtpool.tile([128, IN_F], F32, tag="xt")
            for q in range(NBLK // 4):
                pt = ps_t.tile([128, 512], F32, tag="pt")
                for k in range(4):
                    j = q * 4 + k
                    nc.tensor.transpose(pt[:, k * 128:(k + 1) * 128],
                                        xin[:, j * 128:(j + 1) * 128], ident)
                if q % 2 == 0:
                    nc.vector.tensor_copy(xt[:, q * 512:(q + 1) * 512], pt[:])
                else:
                    nc.scalar.copy(xt[:, q * 512:(q + 1) * 512], pt[:])

            # ---- phase A matmuls: Z = M @ blocks ----
            zre = zpool.tile([128, IN_F], F32, tag="zre")
            zim = zpool.tile([128, IN_F], F32, tag="zim")
            for c in range(8):
                sl = slice(c * 512, (c + 1) * 512)
                pm = ps_m.tile([128, 512], F32, tag="pm")
                nc.tensor.matmul(pm[:], mre_t, xt[:, sl], start=True, stop=True)
                if c % 2 == 0:
                    nc.vector.tensor_copy(zre[:, sl], pm[:])
                else:
                    nc.scalar.copy(zre[:, sl], pm[:])
                pm2 = ps_m.tile([128, 512], F32, tag="pm")
                nc.tensor.matmul(pm2[:], mim_t, xt[:, sl], start=True, stop=True)
                if c % 2 == 0:
                    nc.scalar.copy(zim[:, sl], pm2[:])
                else:
                    nc.vector.tensor_copy(zim[:, sl], pm2[:])

            # ---- butterfly stages 8..12 (in place on zre/zim) ----
            col = 0
            for s in range(8, 13):
                G = 1 << (s - 7)          # group width in blocks
                hb = G // 2
                ng = NBLK // G
                z3r = zre[:].rearrange("p (g c) -> p g c", g=ng)
                z3i = zim[:].rearrange("p (g c) -> p g c", g=ng)
                for jr in range(hb):
                    twr = cw[:, _TWR + col:_TWR + col + 1]
                    twi = cw[:, _TWI + col:_TWI + col + 1]
                    col += 1
                    lo = slice(jr * 128, (jr + 1) * 128)
                    hi = slice((hb + jr) * 128, (hb + jr + 1) * 128)
                    for g0 in range(0, ng, 8):
                        g1 = min(g0 + 8, ng)
                        ngc = g1 - g0
                        L = ngc * 128
                        lo_r, hi_r = z3r[:, g0:g1, lo], z3r[:, g0:g1, hi]
                        lo_i, hi_i = z3i[:, g0:g1, lo], z3i[:, g0:g1, hi]
                        tm1 = spool.tile([128, L], F32, tag="tm1")
                        tm2 = spool.tile([128, L], F32, tag="tm2")
                        tre = spool.tile([128, L], F32, tag="tre")
                        tim = spool.tile([128, L], F32, tag="tim")
                        v1 = tm1[:].rearrange("p (g c) -> p g c", g=ngc)
                        v2 = tm2[:].rearrange("p (g c) -> p g c", g=ngc)
                        vr = tre[:].rearrange("p (g c) -> p g c", g=ngc)
                        vi = tim[:].rearrange("p (g c) -> p g c", g=ngc)
                        nc.scalar.mul(v1, hi_i, twi)
                        nc.scalar.mul(v2, hi_i, twr)
                        nc.vector.scalar_tensor_tensor(
                            vr, hi_r, twr, v1, op0=AL.mult, op1=AL.subtract)
                        nc.vector.scalar_tensor_tensor(
                            vi, hi_r, twi, v2, op0=AL.mult, op1=AL.add)
                        # hi' = lo - t first (reads lo), then lo' += t
                        nc.gpsimd.tensor_tensor(hi_r, lo_r, vr, op=AL.subtract)
                        nc.gpsimd.tensor_tensor(hi_i, lo_i, vi, op=AL.subtract)
                        nc.vector.tensor_tensor(lo_r, lo_r, vr, op=AL.add)
                        nc.vector.tensor_tensor(lo_i, lo_i, vi, op=AL.add)

            # ---- stage 13: t = tw13 * v  (per block j) ----
            tr13 = tpool.tile([128, IN_F], F32, tag="tr13")
            ti13 = tpool.tile([128, IN_F], F32, tag="ti13")
            for j in range(NBLK):
                bs = slice(j * 128, (j + 1) * 128)
                ccol = cw[:, _T13C + j:_T13C + j + 1]
                scol = cw[:, _T13S + j:_T13S + j + 1]
                tm1 = spool.tile([128, 128], F32, tag="tm1")
                tm2 = spool.tile([128, 128], F32, tag="tm2")
                nc.scalar.mul(tm1[:], zim[:, bs], scol)
                nc.scalar.mul(tm2[:], zim[:, bs], ccol)
                nc.vector.scalar_tensor_tensor(
                    tr13[:, bs], zre[:, bs], ccol, tm1[:],
                    op0=AL.mult, op1=AL.subtract)
                nc.vector.scalar_tensor_tensor(
                    ti13[:, bs], zre[:, bs], scol, tm2[:],
                    op0=AL.mult, op1=AL.add)

            # ---- transpose back + interleave re/im, write [t, -t] ----
            for jc in range(NBLK // 4):       # chunks of 4 blocks
                op_ = opool.tile([128, 1024], F32, tag="op")
                on_ = opool.tile([128, 1024], F32, tag="on")
                pr = ps_o.tile([128, 512], F32, tag="por")
                pi = ps_o.tile([128, 512], F32, tag="poi")
                for k in range(4):
                    j = jc * 4 + k
                    bs = slice(j * 128, (j + 1) * 128)
                    nc.tensor.transpose(pr[:, k * 128:(k + 1) * 128],
                                        tr13[:, bs], ident)
                    nc.tensor.transpose(pi[:, k * 128:(k + 1) * 128],
                                        ti13[:, bs], ident)
                vp = op_[:].rearrange("p (f two) -> p f two", two=2)
                vn = on_[:].rearrange("p (f two) -> p f two", two=2)
                if jc % 2 == 0:
                    nc.vector.tensor_copy(vp[:, :, 0], pr[:])
                    nc.scalar.copy(vp[:, :, 1], pi[:])
                    nc.vector.tensor_scalar_mul(vn[:, :, 0], pr[:], -1.0)
                    nc.scalar.mul(vn[:, :, 1], pi[:], -1.0)
                else:
                    nc.scalar.copy(vp[:, :, 0], pr[:])
                    nc.vector.tensor_copy(vp[:, :, 1], pi[:])
                    nc.scalar.mul(vn[:, :, 0], pr[:], -1.0)
                    nc.vector.tensor_scalar_mul(vn[:, :, 1], pi[:], -1.0)
                c0 = jc * 1024
                nc.sync.dma_start(y_d[r0:r0 + 128, c0:c0 + 1024], op_[:])
                nc.sync.dma_start(
                    y_d[r0:r0 + 128, N + c0:N + c0 + 1024], on_[:])

    nc.compile()
    return nc


def kernel(x: np.ndarray, weights: np.ndarray) -> np.ndarray:
    x = np.ascontiguousarray(np.asarray(x, dtype=np.float32))
    w = np.asarray(weights, dtype=np.float32)
    if "nc" not in _CACHE:
        _CACHE["nc"] = _build_program()
    nc = _CACHE["nc"]
    cw = _host_consts(w)
    in_maps = [
        {"x": x[ci * B_CORE:(ci + 1) * B_CORE], "cw": cw}
        for ci in range(NCORES)
    ]
    res = run_bass_kernel_spmd(nc, in_maps, list(range(NCORES)))
    _CACHE["last_results"] = res
    out = np.concatenate([res.results[ci]["y"] for ci in range(NCORES)], axis=0)
    return out.view(np.complex64)


# revision 17
# speedup vs baseline: 1.0230x; 1.0230x over previous
"""Trainium2 Bass kernel for nn_CustomLinear (learned-twiddle butterfly net).

Math (verified vs reference in fp32, rel err ~4e-7):
  reference pads x [2048,4096] to [2048,8192], half-swaps (XOR N/2), then 13
  radix-2 butterfly stages with learned twiddles tw_s[i] = exp(-2j*pi/N *
  k*w[k]), k = i*N/step.  After the half-swap the first 4096 elements of each
  row are ZERO, so:
    - stages 1..7  == one 128x128 complex matrix M applied to each of the 32
      nonzero 128-blocks (input real -> 2 real matmuls)
    - stages 8..12 == butterflies on the 4096-element nonzero subvector
    - stage 13     == out = [t, -t],  t = tw13 * v   (lo half is zero)

Device layout: transposed blocks [e=partition, (j,r)=free] so twiddles are
per-partition scalars (scalar_tensor_tensor / ACT scale).  PE does the block
transposes in/out and the M matmuls; DVE/ACT/GPSIMD share butterflies+copies.

Sharding: pure data parallel, batch 2048 -> 8 cores x 256 rows.
"""
import numpy as np
from contextlib import ExitStack

import concourse.bacc as bacc
import concourse.mybir as mybir
from concourse.tile import TileContext
from concourse.bass_utils import run_bass_kernel_spmd

N = 8192
B = 2048
IN_F = 4096
NCORES = 8
B_CORE = B // NCORES          # 256 rows per core
NTILES = B_CORE // 128        # 2 row-tiles of 128 rows
NBLK = 32                     # nonzero 128-blocks per row
F32 = mybir.dt.float32

# const tensor column layout
_MRE, _MIM, _ID = 0, 128, 256
_TWR, _TWI = 384, 415         # 31 cols each (stages 8..12)
_T13C, _T13S = 446, 478       # 32 cols each
CW_W = 512

_CACHE = {}


def _stage_tw(s, w):
    step = 1 << s
    half = step >> 1
    k = np.arange(half) * (N // step)
    ang = (-2.0 * np.pi / N) * k.astype(np.float64) * w[k].astype(np.float64)
    return np.exp(1j * ang)


def _host_consts(w):
    M = np.eye(128, dtype=np.complex128)
    for s in range(1, 8):
        step = 1 << s
        half = step >> 1
        tw = _stage_tw(s, w)
        Bm = np.zeros((step, step), np.complex128)
        Bm[:half, :half] = np.eye(half)
        Bm[:half, half:] = np.diag(tw)
        Bm[half:, :half] = np.eye(half)
        Bm[half:, half:] = -np.diag(tw)
        M = np.kron(np.eye(128 // step), Bm) @ M

    cw = np.zeros((128, CW_W), np.float32)
    cw[:, _MRE:_MRE + 128] = M.real.T.astype(np.float32)
    cw[:, _MIM:_MIM + 128] = M.imag.T.astype(np.float32)
    cw[:, _ID:_ID + 128] = np.eye(128, dtype=np.float32)
    off = 0
    for s in range(8, 13):
        tw = _stage_tw(s, w)
        hb = 1 << (s - 8)           # hi width in blocks
        for jr in range(hb):
            cw[:, _TWR + off] = tw.real[jr * 128:(jr + 1) * 128]
            cw[:, _TWI + off] = tw.imag[jr * 128:(jr + 1) * 128]
            off += 1
    tw13 = _stage_tw(13, w)
    for j in range(NBLK):
        cw[:, _T13C + j] = tw13.real[j * 128:(j + 1) * 128]
        cw[:, _T13S + j] = tw13.imag[j * 128:(j + 1) * 128]
    return cw


def _build_program():
    nc = bacc.Bacc("TRN2", target_bir_lowering=False, debug=False)
    x_d = nc.dram_tensor("x", [B_CORE, IN_F], F32, kind="ExternalInput").ap()
    cw_d = nc.dram_tensor("cw", [128, CW_W], F32, kind="ExternalInput").ap()
    y_d = nc.dram_tensor("y", [B_CORE, 2 * N], F32, kind="ExternalOutput").ap()

    AL = mybir.AluOpType

    with TileContext(nc) as tc, ExitStack() as ctx:
        cpool = ctx.enter_context(tc.tile_pool(name="const", bufs=1))
        xpool = ctx.enter_context(tc.tile_pool(name="xin", bufs=2))
        xtpool = ctx.enter_context(tc.tile_pool(name="xt", bufs=2))
        tpool = ctx.enter_context(tc.tile_pool(name="t13", bufs=1))
        zpool = ctx.enter_context(tc.tile_pool(name="z", bufs=1))
        spool = ctx.enter_context(tc.tile_pool(name="scr", bufs=3))
        opool = ctx.enter_context(tc.tile_pool(name="out", bufs=4))
        ps_t = ctx.enter_context(tc.tile_pool(name="ps_t", bufs=2, space="PSUM"))
        ps_m = ctx.enter_context(tc.tile_pool(name="ps_m", bufs=2, space="PSUM"))
        ps_o = ctx.enter_context(tc.tile_pool(name="ps_o", bufs=2, space="PSUM"))

        cw = cpool.tile([128, CW_W], F32)
        nc.sync.dma_start(cw[:], cw_d[:])
        ident = cw[:, _ID:_ID + 128]
        mre_t = cw[:, _MRE:_MRE + 128]
        mim_t = cw[:, _MIM:_MIM + 128]

        for ti in range(NTILES):
            r0 = ti * 128
            # ---- load + transpose 32 blocks:  XT[e, j*128+r] ----
            xin = xpool.tile([128, IN_F], F32, tag="xin")
            nc.sync.dma_start(xin[:, :2048], x_d[r0:r0 + 128, :2048])
            nc.sync.dma_start(xin[:, 2048:], x_d[r0:r0 + 128, 2048:])
            xt = xtpool.tile([128, IN_F], F32, tag="xt")
            for q in range(NBLK // 4):
                pt = ps_t.tile([128, 512], F32, tag="pt")
                for k in range(4):
                    j = q * 4 + k
                    nc.tensor.transpose(pt[:, k * 128:(k + 1) * 128],
                                        xin[:, j * 128:(j + 1) * 128], ident)
                nc.scalar.copy(xt[:, q * 512:(q + 1) * 512], pt[:])

            # ---- phase A matmuls: Z = M @ blocks ----
            z2 = zpool.tile([128, 2 * IN_F], F32, tag="z2")
            zre = z2[:, 0:IN_F]
            zim = z2[:, IN_F:2 * IN_F]
            for c in range(8):
                sl = slice(c * 512, (c + 1) * 512)
                pm = ps_m.tile([128, 512], F32, tag="pm")
                nc.tensor.matmul(pm[:], mre_t, xt[:, sl], start=True, stop=True)
                if c % 2 == 0:
                    nc.vector.tensor_copy(zre[:, sl.start:sl.stop], pm[:])
                else:
                    nc.scalar.copy(zre[:, sl.start:sl.stop], pm[:])
                pm2 = ps_m.tile([128, 512], F32, tag="pm")
                nc.tensor.matmul(pm2[:], mim_t, xt[:, sl], start=True, stop=True)
                nc.scalar.copy(zim[:, sl], pm2[:])

            # ---- butterfly stages 8..12 (in place on zre/zim) ----
            col = 0
            for s in range(8, 13):
                G = 1 << (s - 7)          # group width in blocks
                hb = G // 2
                ng = NBLK // G
                z4 = z2[:].rearrange("p (pl g c) -> p pl g c", pl=2, g=ng)
                z3r = z4[:, 0]
                z3i = z4[:, 1]
                for jr in range(hb):
                    twr = cw[:, _TWR + col:_TWR + col + 1]
                    twi = cw[:, _TWI + col:_TWI + col + 1]
                    col += 1
                    lo = slice(jr * 128, (jr + 1) * 128)
                    hi = slice((hb + jr) * 128, (hb + jr + 1) * 128)
                    for g0 in range(0, ng, 8):
                        g1 = min(g0 + 8, ng)
                        ngc = g1 - g0
                        L = ngc * 128
                        lo_r, hi_r = z3r[:, g0:g1, lo], z3r[:, g0:g1, hi]
                        lo_i, hi_i = z3i[:, g0:g1, lo], z3i[:, g0:g1, hi]
                        tm1 = spool.tile([128, L], F32, tag="tm1")
                        tm2 = spool.tile([128, L], F32, tag="tm2")
                        t2 = spool.tile([128, 2 * L], F32, tag="t2")
                        t4 = t2[:].rearrange("p (pl g c) -> p pl g c",
                                             pl=2, g=ngc)
                        v1 = tm1[:].rearrange("p (g c) -> p g c", g=ngc)
                        v2 = tm2[:].rearrange("p (g c) -> p g c", g=ngc)
                        vr, vi = t4[:, 0], t4[:, 1]
                        nc.scalar.mul(v1, hi_i, twi)
                        nc.scalar.mul(v2, hi_i, twr)
                        nc.vector.scalar_tensor_tensor(
                            vr, hi_r, twr, v1, op0=AL.mult, op1=AL.subtract)
                        nc.vector.scalar_tensor_tensor(
                            vi, hi_r, twi, v2, op0=AL.mult, op1=AL.add)
                        # merged-plane combines: hi' = lo - t (reads lo first),
                        # then lo' += t  -- one op covers re+im planes
                        lo_b = z4[:, :, g0:g1, lo]
                        hi_b = z4[:, :, g0:g1, hi]
                        t4g = t4[:, :, :, :]
                        nc.gpsimd.tensor_tensor(hi_b, lo_b, t4g, op=AL.subtract)
                        nc.vector.tensor_tensor(lo_b, lo_b, t4g, op=AL.add)

            # ---- stage 13: t = tw13 * v  (per block j) ----
            tr13 = tpool.tile([128, IN_F], F32, tag="tr13")
            ti13 = tpool.tile([128, IN_F], F32, tag="ti13")
            for j in range(NBLK):
                bs = slice(j * 128, (j + 1) * 128)
                ccol = cw[:, _T13C + j:_T13C + j + 1]
                scol = cw[:, _T13S + j:_T13S + j + 1]
                tm1 = spool.tile([128, 128], F32, tag="tm1")
                tm2 = spool.tile([128, 128], F32, tag="tm2")
                nc.scalar.mul(tm1[:], zim[:, bs], scol)
                nc.scalar.mul(tm2[:], zim[:, bs], ccol)
                nc.vector.scalar_tensor_tensor(
                    tr13[:, bs], zre[:, bs], ccol, tm1[:],
                    op0=AL.mult, op1=AL.subtract)
                nc.vector.scalar_tensor_tensor(
                    ti13[:, bs], zre[:, bs], scol, tm2[:],
                    op0=AL.mult, op1=AL.add)

            # ---- transpose back + interleave re/im, write [t, -t] ----
            for jc in range(NBLK // 4):       # chunks of 4 blocks
                op_ = opool.tile([128, 1024], F32, tag="op")
                on_ = opool.tile([128, 1024], F32, tag="on")
                pr = ps_o.tile([128, 512], F32, tag="por")
                pi = ps_o.tile([128, 512], F32, tag="poi")
                for k in range(4):
                    j = jc * 4 + k
                    bs = slice(j * 128, (j + 1) * 128)
                    nc.tensor.transpose(pr[:, k * 128:(k + 1) * 128],
                                        tr13[:, bs], ident)
                    nc.tensor.transpose(pi[:, k * 128:(k + 1) * 128],
                                        ti13[:, bs], ident)
                vp = op_[:].rearrange("p (f two) -> p f two", two=2)
                vn = on_[:].rearrange("p (f two) -> p f two", two=2)
                if jc % 2 == 0:
                    nc.vector.tensor_copy(vp[:, :, 0], pr[:])
                    nc.scalar.copy(vp[:, :, 1], pi[:])
                    nc.vector.tensor_scalar_mul(vn[:, :, 0], pr[:], -1.0)
                    nc.scalar.mul(vn[:, :, 1], pi[:], -1.0)
                else:
                    nc.scalar.copy(vp[:, :, 0], pr[:])
                    nc.vector.tensor_copy(vp[:, :, 1], pi[:])
                    nc.scalar.mul(vn[:, :, 0], pr[:], -1.0)
                    nc.vector.tensor_scalar_mul(vn[:, :, 1], pi[:], -1.0)
                c0 = jc * 1024
                nc.sync.dma_start(y_d[r0:r0 + 128, c0:c0 + 1024], op_[:])
                nc.sync.dma_start(
                    y_d[r0:r0 + 128, N + c0:N + c0 + 1024], on_[:])

    nc.compile()
    return nc


def kernel(x: np.ndarray, weights: np.ndarray) -> np.ndarray:
    x = np.ascontiguousarray(np.asarray(x, dtype=np.float32))
    w = np.asarray(weights, dtype=np.float32)
    if "nc" not in _CACHE:
        _CACHE["nc"] = _build_program()
    nc = _CACHE["nc"]
    cw = _host_consts(w)
    in_maps = [
        {"x": x[ci * B_CORE:(ci + 1) * B_CORE], "cw": cw}
        for ci in range(NCORES)
    ]
    res = run_bass_kernel_spmd(nc, in_maps, list(range(NCORES)))
    _CACHE["last_results"] = res
    out = np.concatenate([res.results[ci]["y"] for ci in range(NCORES)], axis=0)
    return out.view(np.complex64)
